# revision 10
# baseline (speedup 1.0000x reference)
"""Contrastive loss kernel for Trainium2 (8 NeuronCores, Bass/Tile).

v3 strategy (data-parallel over rows of embeddings1, fp8 DoubleRow matmul):
  - Host normalizes both embedding sets (f64), computes the diagonal logits
    exactly, scales by 16 and quantizes to fp8e4m3.  Each core gets its own
    512 normalized e1 rows (transposed, k-slab layout) plus all of e2
    (transposed, k-slab layout).
  - Core c computes its [512, 4096] logit tile  S[i, j] = 256*<e1n_i, e2n_j>
    with fp8 DoubleRow matmuls (contraction 2x128 per step, 4 steps for
    D=1024).  Loop order (ib, jc-group, q, jc) keeps one stationary operand
    across 4 moving matmuls so LDWEIGHTS stays hidden.
  - ACT applies exp(0.0390625 * psum) (= exp(10 * cos sim)), writing fp8
    exps to SBUF; the free ACT accum gives per-i partial row sums.
  - Column partials (over the core's 512 i) come from 16 DoubleRow matmuls
    with one-hot mask weights routing j-chunk jc onto PSUM partition jc of
    a single [8, 512] accumulator.
  - HAM warmup: 8 junk matmuls on a memset tile (no DMA dependency) flip
    the PE clock gate to 8/8 while e2 streams in.

Outputs per core: rows [128, 32] (accum partials, i = ib*128+p, col ib*8+jc),
colp [8, 512] (column partials, j = jc*512 + col).
"""

import os
import sys

import numpy as np

for _p in ("/root/.axon_site", "/root/.axon_site/_ro/trn_rl_repo",
           "/root/.axon_site/_ro/pypackages", "/opt/trn_rl_repo"):
    if os.path.isdir(_p) and _p not in sys.path:
        sys.path.append(_p)

import ml_dtypes

N, D = 4096, 1024
NCORES = 8
CH = N // NCORES          # 512 rows of e1 per core
KT = D // 128             # 8 contraction subtiles
IBT = CH // 128           # 4 i-blocks per core
JCW = 512                 # j chunk width (one PSUM bank)
JCT = N // JCW            # 8 j chunks
SC = 16.0                 # fp8 pre-scale; logits = psum * 10/SC^2
ACT_SCALE = 10.0 / (SC * SC)

_CACHE = {}


def _legalize_waits(nc, cap=1):
    """Split >cap semaphore waits per instruction onto preceding NOPs."""
    import concourse.mybir as mybir
    nid = 0
    for f in nc.m.functions:
        for b in f.blocks:
            insts = b.instructions
            i = 0
            while i < len(insts):
                inst = insts[i]
                si = inst.sync_info
                if si is not None and si.on_wait and len(si.on_wait) > cap:
                    waits = list(si.on_wait)
                    inst.sync_info = mybir.SyncInfo(
                        on_wait=waits[-cap:], on_update=list(si.on_update))
                    excess = waits[:-cap]
                    pos = i
                    for j in range(0, len(excess), cap):
                        nop = mybir.InstNoOp(
                            name=f"I-waitnop-{nid}", ins=[], outs=[])
                        nid += 1
                        nop.engine = inst.engine
                        nop.sync_info = mybir.SyncInfo(
                            on_wait=excess[j:j + cap], on_update=[])
                        insts.insert(pos, nop)
                        pos += 1
                        i += 1
                i += 1
    return nc


def build_nc(legalize=True):
    import concourse.bass as bass
    import concourse.mybir as mybir
    import concourse.tile as tile
    from contextlib import ExitStack

    fp32 = mybir.dt.float32
    fp8 = mybir.dt.float8e4
    AF = mybir.ActivationFunctionType
    DR = mybir.MatmulPerfMode.DoubleRow

    nc = bass.Bass(trn_type="TRN2")
    e1t_d = nc.dram_tensor("e1t", [128, KT * CH], fp8, kind="ExternalInput")
    e2t_d = nc.dram_tensor("e2t", [KT, 128, N], fp8, kind="ExternalInput")
    rows_d = nc.dram_tensor("rows", [128, IBT * JCT], fp32,
                            kind="ExternalOutput")
    colp_d = nc.dram_tensor("colp", [JCT, JCW], fp32, kind="ExternalOutput")

    with ExitStack() as ctx:
        tc = ctx.enter_context(tile.TileContext(nc))
        res = ctx.enter_context(tc.tile_pool(name="res", bufs=1))
        pmm = ctx.enter_context(tc.tile_pool(name="pmm", bufs=7, space="PSUM"))
        pcol = ctx.enter_context(tc.tile_pool(name="pcol", bufs=1,
                                              space="PSUM"))

        e2t_sb = res.tile([128, KT, N], fp8)        # 32 KiB/part
        e1t_sb = res.tile([128, KT, CH], fp8)       # 4 KiB/part
        exps_sb = res.tile([128, IBT, N], fp8)      # 16 KiB/part
        rows_sb = res.tile([128, IBT * JCT], fp32)
        colp_sb = res.tile([JCT, JCW], fp32)
        jnk = res.tile([128, 2, JCW], fp8)          # warmup operand
        # mask4d[:, :, jc, :] = [128, 2, 8] DR pair with column jc all-ones:
        # routes the ones-matmul partial for j-chunk jc onto PSUM partition jc.
        mask4d = res.tile([128, 2, JCT, JCT], fp8)
        nc.vector.memset(jnk, 0.0)
        nc.vector.memset(mask4d, 0.0)
        for jc in range(JCT):
            nc.vector.memset(mask4d[:, 0, jc, jc:jc + 1], 1.0)
            nc.vector.memset(mask4d[:, 1, jc, jc:jc + 1], 1.0)

        # ---- HAM warmup: junk DR matmuls, no DMA dependency ----
        for w in range(8):
            pj = pmm.tile([128, JCW], fp32, tag="pl")
            nc.tensor.matmul(pj, lhsT=jnk[:, :, 0:128], rhs=jnk[:, :, :],
                             start=True, stop=True, perf_mode=DR)

        # ---- input DMAs, k-slab order ----
        nc.sync.dma_start(out=e1t_sb[:, :, :], in_=e1t_d[:, :])
        for q in range(KT // 2):
            nc.sync.dma_start(out=e2t_sb[:, 2 * q:2 * q + 2, :],
                              in_=e2t_d[2 * q:2 * q + 2])

        # ---- main: S[i, j] tiles, exp, row-sum accum ----
        # Column partials (masked-ones^T @ exps into one [8, 512] PSUM) are
        # emitted per i-block pair, right after the pair's ACTs, so only the
        # last 8 ride the tail.
        pc = pcol.tile([JCT, JCW], fp32, tag="pc")
        JG = 4                      # j-chunks per stationary group
        for ib in range(IBT):
            isl = slice(ib * 128, (ib + 1) * 128)
            for jg in range(JCT // JG):
                pls = []
                for q in range(KT // 2):
                    lhsT = e1t_sb[:, 2 * q:2 * q + 2, isl]
                    for j4 in range(JG):
                        jc = jg * JG + j4
                        jsl = slice(jc * JCW, (jc + 1) * JCW)
                        if q == 0:
                            pls.append(pmm.tile([128, JCW], fp32, tag="pl",
                                                name=f"pl_{ib}_{jg}_{j4}"))
                        nc.tensor.matmul(pls[j4], lhsT=lhsT,
                                         rhs=e2t_sb[:, 2 * q:2 * q + 2, jsl],
                                         start=(q == 0),
                                         stop=(q == KT // 2 - 1),
                                         perf_mode=DR)
                for j4 in range(JG):
                    jc = jg * JG + j4
                    jsl = slice(jc * JCW, (jc + 1) * JCW)
                    idx = ib * JCT + jc
                    nc.scalar.activation(out=exps_sb[:, ib, jsl], in_=pls[j4],
                                         func=AF.Exp, scale=ACT_SCALE,
                                         accum_out=rows_sb[:, idx:idx + 1])
            if ib % 2 == 1:
                a = ib // 2
                for jc in range(JCT):
                    jsl = slice(jc * JCW, (jc + 1) * JCW)
                    nc.tensor.matmul(pc, lhsT=mask4d[:, :, jc, :],
                                     rhs=exps_sb[:, 2 * a:2 * a + 2, jsl],
                                     start=(a == 0 and jc == 0),
                                     stop=(a == IBT // 2 - 1 and jc == JCT - 1),
                                     perf_mode=DR)

        nc.sync.dma_start(out=rows_d[:, :], in_=rows_sb)
        nc.vector.tensor_copy(out=colp_sb, in_=pc)
        nc.sync.dma_start(out=colp_d[:, :], in_=colp_sb)
    return _legalize_waits(nc) if legalize else nc


def _get_nc():
    if "nc" not in _CACHE:
        _CACHE["nc"] = build_nc()
    return _CACHE["nc"]


def _run(in_maps, trace=False, **kw):
    from concourse.bass_utils import run_bass_kernel_spmd
    return run_bass_kernel_spmd(_get_nc(), in_maps,
                                core_ids=list(range(NCORES)),
                                trace=trace, **kw)


def _prep(embeddings1, embeddings2):
    e1 = np.asarray(embeddings1, dtype=np.float64)
    e2 = np.asarray(embeddings2, dtype=np.float64)
    e1n = e1 / np.maximum(np.linalg.norm(e1, axis=1, keepdims=True), 1e-12)
    e2n = e2 / np.maximum(np.linalg.norm(e2, axis=1, keepdims=True), 1e-12)
    ldiag = 10.0 * np.einsum("nd,nd->n", e1n, e2n)
    fp8 = ml_dtypes.float8_e4m3
    q1 = (e1n * SC).astype(np.float32).astype(fp8)
    q2 = (e2n * SC).astype(np.float32).astype(fp8)
    # e2: k-slab layout [KT, 128, N]; e1: SBUF layout [128, KT*CH]
    e2t = np.ascontiguousarray(q2.T.reshape(KT, 128, N))
    e1ts = []
    for c in range(NCORES):
        sl = slice(c * CH, (c + 1) * CH)
        p = q1[sl].T.reshape(KT, 128, CH).transpose(1, 0, 2)
        e1ts.append(np.ascontiguousarray(p.reshape(128, KT * CH)))
    return e1ts, e2t, ldiag


def kernel(embeddings1, embeddings2, _trace=False, _full_result=False):
    e1ts, e2t, ldiag = _prep(embeddings1, embeddings2)
    in_maps = [{"e1t": e1ts[c], "e2t": e2t} for c in range(NCORES)]
    bres = _run(in_maps, trace=_trace)
    outs = bres.results

    rows = np.empty(N, dtype=np.float64)
    colsum = np.zeros(N, dtype=np.float64)
    for c, o in enumerate(outs):
        r = np.asarray(o["rows"], dtype=np.float64)  # [128, IBT*JCT]
        for ib in range(IBT):
            part = r[:, ib * JCT:(ib + 1) * JCT].sum(axis=1)
            rows[c * CH + ib * 128:c * CH + (ib + 1) * 128] = part
        colsum += np.asarray(o["colp"], dtype=np.float64).reshape(-1)

    ed = np.exp(ldiag)
    row_denom = rows - ed
    col_denom = colsum - ed
    sim12 = float(np.sum(ldiag - np.log(row_denom)))
    sim21 = float(np.sum(ldiag - np.log(col_denom)))
    result = (np.float32(-sim12), np.float32(-sim21))
    if _full_result:
        return result, bres
    return result
